# revision 14
# baseline (speedup 1.0000x reference)
"""Multi-head attention (B=1, L=2048, D=1024, H=16) on 8 TRN2 NeuronCores.

Sharding: tensor-parallel over heads. Core i computes heads 2i, 2i+1:
  - projections with column shards of w_q/w_k/w_v (128 cols each)
  - full attention for its 2 heads
  - partial output projection with the matching 128-row shard of w_o
Host sums the 8 partial outputs and adds the fused bias b_o + b_v @ w_o
(b_v contributes a constant row to the output; b_k cancels in softmax).

Strip-pipelined schedule (all matmuls bf16, fp32 PSUM):
  - q processed in 4 strips of 512; per (strip, kt) iteration:
      S^T pair (row-tiled K=64 matmuls, heads at PE row groups 0/64)
      -> ONE exp over [128, 1024] (both heads, single PSUM tile)
      -> AV pair (col-tiled M=64, heads at PSUM partition groups 0/64)
      -> denominator accumulate split across VectorE / GpSimd by kt parity
  - tensor queue padded with out-projection chunks of strip s-1,
    q-projection of strip s+1, vh blocks, and normalize matmuls so the
    PE never idles (keeps the 2.4 GHz p-state)
  - host supplies q strip-major and v kt-major so DMA descriptors stay
    large and vh blocks become available incrementally
  - per-strip denominator reciprocal via partition-spread DMA
"""

import os
import numpy as np
import ml_dtypes

import concourse.bass as bass
import concourse.mybir as mybir
import concourse.tile as tile
from concourse import bacc
from concourse.bass import ts
from concourse.bass_utils import run_bass_kernel_spmd

P = 128
L = 2048
D = 1024
DH = 64
NCORES = 8
NSTRIP = 4
SW = 512  # strip width (q columns per strip)
KT = D // P  # 8 contraction tiles for the projections
LT = L // P  # 16 seq tiles
BF16 = mybir.dt.bfloat16
F32 = mybir.dt.float32
AF = mybir.ActivationFunctionType
ALU = mybir.AluOpType

TRACE = False  # test.py flips this to get an NTFF profile / exec_time_ns
LAST_RESULT = {}

_CACHED_NC = None


def _build():
    nc = bacc.Bacc("TRN2", target_bir_lowering=False, debug=False, num_devices=NCORES)

    kS = nc.dram_tensor("kS", [NSTRIP, P, KT, SW], BF16, kind="ExternalInput")
    qS = nc.dram_tensor("qS", [NSTRIP, P, KT, SW], BF16, kind="ExternalInput")
    vK = nc.dram_tensor("vK", [LT, P, KT, P], BF16, kind="ExternalInput")
    wq = nc.dram_tensor("wq", [P, KT, P], BF16, kind="ExternalInput")
    wk = nc.dram_tensor("wk", [P, KT, P], BF16, kind="ExternalInput")
    wv = nc.dram_tensor("wv", [P, KT, P], BF16, kind="ExternalInput")
    bq = nc.dram_tensor("bq", [P, 1], F32, kind="ExternalInput")
    wo = nc.dram_tensor("wo", [P, D], BF16, kind="ExternalInput")
    out = nc.dram_tensor("out", [P, LT, D], BF16, kind="ExternalOutput")

    with tile.TileContext(nc) as tc:
        with (
            tc.tile_pool(name="const", bufs=1) as const_pool,
            tc.tile_pool(name="inputs", bufs=1) as in_pool,
            tc.tile_pool(name="proj", bufs=1) as proj_pool,
            tc.tile_pool(name="work", bufs=1) as work_pool,
            tc.tile_pool(name="ps", bufs=1, space="PSUM") as psp,
            tc.tile_pool(name="ptp", bufs=3) as pt_pool,
            tc.tile_pool(name="accp", bufs=2) as acc_pool,
            tc.tile_pool(name="up", bufs=2) as u_pool,
            tc.tile_pool(name="osbp", bufs=2) as osb_pool,
        ):
            ones_c = const_pool.tile([P, P], BF16)
            nc.vector.memset(ones_c[:], 1.0)
            scr = const_pool.tile([1, 32], F32)
            nc.scalar.activation(scr[:], ones_c[0:1, 0:32], AF.Exp)

            # ---- input DMAs: weights, then k, then q strip0, v blocks,
            # then remaining q strips ----
            wq_sb = in_pool.tile([P, KT, P], BF16)
            wk_sb = in_pool.tile([P, KT, P], BF16)
            wv_sb = in_pool.tile([P, KT, P], BF16)
            bq_sb = in_pool.tile([P, 1], F32)
            wo_sb = in_pool.tile([P, D], BF16)
            nc.sync.dma_start(wk_sb[:], wk[:])
            nc.scalar.dma_start(wq_sb[:], wq[:])
            nc.gpsimd.dma_start(wv_sb[:], wv[:])
            nc.scalar.dma_start(bq_sb[:], bq[:])
            nc.gpsimd.dma_start(wo_sb[:], wo[:])

            kS_sb = in_pool.tile([P, NSTRIP, KT, SW], BF16)
            qS_sb = in_pool.tile([P, NSTRIP, KT, SW], BF16)
            vK_sb = in_pool.tile([P, LT, KT, P], BF16)
            # minimal upfront preamble: kS0/kS1, qS0, vK0-4; everything
            # else is released later, paced by the scalar queue's exps
            nc.sync.dma_start(kS_sb[:, 0, 0:4, :], kS[0][:, 0:4, :])
            nc.gpsimd.dma_start(kS_sb[:, 0, 4:8, :], kS[0][:, 4:8, :])
            nc.scalar.dma_start(qS_sb[:, 0, :, :], qS[0])
            nc.gpsimd.dma_start(kS_sb[:, 1, :, :], kS[1])
            nc.sync.dma_start(vK_sb[:, 0, :, :], vK[0])
            nc.scalar.dma_start(vK_sb[:, 1, :, :], vK[1])
            nc.sync.dma_start(vK_sb[:, 2, :, :], vK[2])
            nc.scalar.dma_start(vK_sb[:, 3, :, :], vK[3])
            nc.sync.dma_start(vK_sb[:, 4, :, :], vK[4])
            nc.scalar.dma_start(vK_sb[:, 5, :, :], vK[5])
            nc.sync.dma_start(vK_sb[:, 6, :, :], vK[6])

            def paced_dma(s, kt):
                """Deferred input DMAs, triggered on the scalar queue right
                after exp(s, kt) so the stream is paced by loop progress."""
                if s == 0:
                    if kt <= 8:
                        nc.scalar.dma_start(vK_sb[:, kt + 7, :, :], vK[kt + 7])
                    if kt == 0:
                        nc.scalar.dma_start(kS_sb[:, 2, :, :], kS[2])
                    elif kt == 1:
                        nc.scalar.dma_start(kS_sb[:, 3, :, :], kS[3])
                    elif kt == 2:
                        nc.scalar.dma_start(qS_sb[:, 1, :, :], qS[1])
                elif s < NSTRIP - 1 and kt == 1:
                    nc.scalar.dma_start(qS_sb[:, s + 1, :, :], qS[s + 1])

            # ---- persistent SBUF tensors ----
            khT = proj_pool.tile([P, L], BF16)
            qhT = proj_pool.tile([P, L], BF16)
            vh_sb = proj_pool.tile([P, LT, P], BF16)  # [kseq, kt, dh-pair]
            lhsT_c = work_pool.tile([P, L], BF16)  # normalized concat^T
            rdb = work_pool.tile([1, NSTRIP, 2 * SW], BF16)  # 1/denominators

            def proj_k_chunk(n):
                """khT[:, n*512:(n+1)*512] (no bias: b_k cancels in softmax)."""
                ps = psp.tile([P, SW], F32, tag="mm", bufs=2, name=f"mmk_{n}")
                for t in range(KT):
                    nc.tensor.matmul(
                        ps[:],
                        wk_sb[:, t, :],
                        kS_sb[:, n, t, :],
                        start=(t == 0),
                        stop=(t == KT - 1),
                    )
                nc.scalar.copy(khT[:, ts(n, SW)], ps[:])

            _qps = {}

            def proj_q_part(s, half):
                """Half of the q-projection for strip s (pad-slot sized)."""
                if half == 0:
                    _qps[s] = psp.tile(
                        [P, SW], F32, tag="mm", bufs=2, name=f"mmq_{s}"
                    )
                ps = _qps[s]
                for t in range(4 * half, 4 * half + 4):
                    nc.tensor.matmul(
                        ps[:],
                        wq_sb[:, t, :],
                        qS_sb[:, s, t, :],
                        start=(t == 0),
                        stop=(t == KT - 1),
                    )
                if half == 1:
                    nc.vector.tensor_scalar(
                        qhT[:, ts(s, SW)], ps[:], bq_sb[:], None, op0=ALU.add
                    )

            def proj_q_strip(s):
                proj_q_part(s, 0)
                proj_q_part(s, 1)

            def vh_block(b):
                """vh_sb[:, b, :] = (v @ w_v)[b-th kseq tile] directly."""
                ps = psp.tile([P, SW], F32, tag="mm", bufs=2, name=f"mmv_{b}")
                for t in range(KT):
                    nc.tensor.matmul(
                        ps[:, 0:P],
                        vK_sb[:, b, t, :],
                        wv_sb[:, t, :],
                        start=(t == 0),
                        stop=(t == KT - 1),
                    )
                nc.vector.tensor_copy(vh_sb[:, b, :], ps[:, 0:P])

            # ---- prologue ----
            proj_k_chunk(0)
            proj_q_strip(0)

            # ---- strip-pipelined attention ----
            accs = [None] * NSTRIP
            us = [None] * NSTRIP
            dsps = [None] * NSTRIP
            osbs = {}

            def fin_a(s):
                """Denominator column-sums of strip s + spread DMA."""
                acc_g, acc_v = accs[s]
                nc.vector.tensor_tensor(acc_v[:], acc_v[:], acc_g[:], op=ALU.add)
                dps = psp.tile([P, SW], F32, tag="mm", bufs=2, name=f"dcs_{s}")
                for h in (0, 1):
                    nc.tensor.matmul(
                        dps[32 * h : 32 * h + 1, :],
                        ones_c[:, 0:1],
                        acc_v[:, ts(h, SW)],
                    )
                dsb = work_pool.tile(
                    [1, 2 * SW], F32, tag="dsb", bufs=2, name=f"dsb_{s}"
                )
                nc.scalar.copy(dsb[0:1, 0:SW], dps[0:1, :])
                nc.scalar.copy(dsb[0:1, SW : 2 * SW], dps[32:33, :])
                dsp = work_pool.tile([P, 8], F32, tag="dsp", bufs=2, name=f"dsp_{s}")
                dsps[s] = dsp
                nc.sync.dma_start(dsp[0:DH, :], dsb[0:1, 0:SW])
                nc.gpsimd.dma_start(dsp[DH:P, :], dsb[0:1, SW : 2 * SW])

            def fin_b(s):
                """Reciprocal on the spread layout + gather back."""
                dsp = dsps[s]
                nc.vector.reciprocal(dsp[:], dsp[:])
                dspb = work_pool.tile([P, 8], BF16, tag="dspb", bufs=2, name=f"dspb_{s}")
                nc.vector.tensor_copy(dspb[:], dsp[:])
                nc.sync.dma_start(rdb[0:1, s, 0:SW], dspb[0:DH, :])
                nc.gpsimd.dma_start(rdb[0:1, s, SW : 2 * SW], dspb[DH:P, :])

            def fin_c(s):
                """Broadcast 1/d over partitions, normalize -> lhsT_c."""
                bc = psp.tile([P, SW], F32, tag="mm", bufs=2, name=f"bc_{s}")
                for h in (0, 1):
                    nc.tensor.matmul(
                        bc[ts(h, DH), :],
                        ones_c[0:1, 0:DH],
                        rdb[0:1, s, ts(h, SW)],
                    )
                nc.vector.tensor_tensor(
                    lhsT_c[:, ts(s, SW)], us[s][:], bc[:], op=ALU.mult
                )

            def outproj_chunk(s, m, n, cp_eng="v"):
                """Partial out-projection chunk (q subtile m, D half n) of
                strip s into the per-strip osb block; one big DMA at the
                end of each strip (8 KB descriptors via the blocked out)."""
                ps = psp.tile([P, SW], F32, tag="mm", bufs=2, name=f"op_{s}_{m}_{n}")
                nc.tensor.matmul(
                    ps[:], lhsT_c[:, ts(4 * s + m, P)], wo_sb[:, ts(n, SW)]
                )
                osb = osbs.get(s)
                if osb is None:
                    osb = osb_pool.tile([P, 4, D], BF16, tag="osb", name=f"osb_{s}")
                    osbs[s] = osb
                (nc.scalar.copy if cp_eng == "s" else nc.vector.tensor_copy)(
                    osb[:, m, ts(n, SW)], ps[:]
                )
                if m == 3 and n == 1:
                    (nc.sync if s % 2 == 0 else nc.gpsimd).dma_start(
                        out[:, 4 * s : 4 * s + 4, :], osb[:]
                    )

            for s in range(NSTRIP):
                av = psp.tile([P, SW], F32, tag="av", bufs=2, name=f"av_{s}")
                acc_g = acc_pool.tile([P, 2 * SW], BF16, tag="accg", name=f"accg_{s}")
                acc_v = acc_pool.tile([P, 2 * SW], BF16, tag="accv", name=f"accv_{s}")
                accs[s] = (acc_g, acc_v)
                pts = [None, None, None]
                for kt in range(LT):
                    # scores: both heads into one PSUM tile (row groups 0/64)
                    st = psp.tile(
                        [P, 2 * SW], F32, tag="st", bufs=2, name=f"st_{s}_{kt}"
                    )
                    reps = 2 if (s > 0 and kt in (1, 2, 14, 15)) else 1
                    for _rep in range(reps):
                        for h in (0, 1):
                            nc.tensor.matmul(
                                st[:, ts(h, SW)],
                                khT[ts(h, DH), ts(kt, P)],
                                qhT[ts(h, DH), ts(s, SW)],
                            )
                    # one exp for both heads; scale 1/sqrt(64) folded in
                    pt = pt_pool.tile([P, 2 * SW], BF16, tag="pt", name=f"pt_{s}_{kt}")
                    pts[kt % 3] = pt
                    nc.scalar.activation(pt[:], st[:], AF.Exp, scale=0.125)
                    paced_dma(s, kt)

                    # tensor pad work (keeps PE busy while exp(kt) finishes)
                    if s == 0:
                        if kt == 0:
                            vh_block(0)
                            vh_block(1)
                            vh_block(2)
                        elif kt <= 13:
                            vh_block(kt + 2)
                        elif kt == 14:
                            proj_q_part(1, 0)
                        elif kt == 15:
                            proj_q_part(1, 1)
                        if kt == 1:
                            proj_k_chunk(1)
                        elif kt == 5:
                            proj_k_chunk(2)
                        elif kt == 9:
                            proj_k_chunk(3)
                    else:
                        if kt == 0:
                            fin_a(s - 1)
                        elif kt == 3:
                            fin_b(s - 1)
                        elif kt == 5:
                            fin_c(s - 1)
                        elif 6 <= kt <= 13:
                            outproj_chunk(s - 1, (kt - 6) // 2, (kt - 6) % 2)
                        elif kt == 14 and s + 1 < NSTRIP:
                            proj_q_part(s + 1, 0)
                        elif kt == 15 and s + 1 < NSTRIP:
                            proj_q_part(s + 1, 1)

                    # AV pair of the previous kt (exp long since done)
                    def av_pair(k):
                        for h in (0, 1):
                            nc.tensor.matmul(
                                av[ts(h, DH), :],
                                vh_sb[:, k, ts(h, DH)],
                                pts[k % 3][:, ts(h, SW)],
                                start=(k == 0),
                                stop=(k == LT - 1),
                            )

                    if kt > 0:
                        av_pair(kt - 1)
                    # two independent denominator chains; gpsimd (slow per
                    # op) only gets mid-strip slots so it never lags fin_a
                    if kt in (2, 5, 8, 11):
                        if kt == 2:
                            nc.gpsimd.tensor_copy(acc_g[:], pt[:])
                        else:
                            nc.gpsimd.tensor_tensor(
                                acc_g[:], acc_g[:], pt[:], op=ALU.add
                            )
                    else:
                        if kt == 0:
                            nc.vector.tensor_copy(acc_v[:], pt[:])
                        else:
                            nc.vector.tensor_tensor(
                                acc_v[:], acc_v[:], pt[:], op=ALU.add
                            )
                av_pair(LT - 1)
                # unnormalized attention out of this strip -> SBUF
                u = u_pool.tile([P, SW], F32, tag="u", name=f"u_{s}")
                us[s] = u
                nc.scalar.copy(u[:], av[:])

            # ---- epilogue for the last strip ----
            s = NSTRIP - 1
            fin_a(s)
            fin_b(s)
            fin_c(s)
            for m in range(4):
                for n in range(2):
                    outproj_chunk(s, m, n, cp_eng="s" if (2 * m + n) % 2 else "v")

    nc.compile()
    return nc


def kernel(q, k, v, w_q, b_q, w_k, b_k, w_v, b_v, w_o, b_o):
    global _CACHED_NC, LAST_RESULT
    if _CACHED_NC is None:
        _CACHED_NC = _build()
    nc = _CACHED_NC

    bf16 = ml_dtypes.bfloat16

    def tile_T(x):  # [L, D] -> [128, D//128, L] contiguous
        xt = np.asarray(x, np.float32)[0].T  # [D, L]
        return np.ascontiguousarray(
            xt.reshape(D // P, P, L).transpose(1, 0, 2)
        ).astype(bf16)

    def tile_w(w):  # [D, 128] -> [128, D//128, 128] contiguous
        return np.ascontiguousarray(
            w.reshape(D // P, P, P).transpose(1, 0, 2)
        ).astype(bf16)

    # k and q strip-major: [NSTRIP, 128, KT, 512]
    k2 = np.ascontiguousarray(
        tile_T(k).reshape(P, KT, NSTRIP, SW).transpose(2, 0, 1, 3)
    )
    q2 = np.ascontiguousarray(
        tile_T(q).reshape(P, KT, NSTRIP, SW).transpose(2, 0, 1, 3)
    )
    # v kt-major: [LT, 128, KT, 128]
    v2 = np.ascontiguousarray(
        tile_T(v).reshape(P, KT, LT, P).transpose(2, 0, 1, 3)
    )
    w_q = np.asarray(w_q, np.float32)
    w_k = np.asarray(w_k, np.float32)
    w_v = np.asarray(w_v, np.float32)
    w_o = np.asarray(w_o, np.float32)
    b_q = np.asarray(b_q, np.float32)
    b_v = np.asarray(b_v, np.float32)
    b_o = np.asarray(b_o, np.float32)

    in_maps = []
    for i in range(NCORES):
        sl = slice(P * i, P * (i + 1))
        in_maps.append(
            {
                "kS": k2,
                "qS": q2,
                "vK": v2,
                "wq": tile_w(w_q[:, sl]),
                "wk": tile_w(w_k[:, sl]),
                "wv": tile_w(w_v[:, sl]),
                "bq": np.ascontiguousarray(b_q[sl]).reshape(P, 1),
                "wo": np.ascontiguousarray(w_o[sl, :]).astype(bf16),
            }
        )

    kwargs = {}
    if TRACE:
        import shutil

        tdir = "/tmp/bass_trace"
        shutil.rmtree(tdir, ignore_errors=True)
        os.makedirs(tdir, exist_ok=True)
        kwargs["tmpdir"] = tdir
    res = run_bass_kernel_spmd(nc, in_maps, list(range(NCORES)), trace=TRACE, **kwargs)
    LAST_RESULT = {
        "exec_time_ns": res.exec_time_ns,
        "trace_path": (res.instructions_and_trace or (None, None))[1],
    }
    acc = np.zeros((L, D), np.float64)
    for i in range(NCORES):
        ob = res.results[i]["out"]  # [P, LT, D] blocked
        acc += ob.transpose(1, 0, 2).reshape(L, D).astype(np.float64)
    # b_k cancels in softmax; b_v and b_o contribute a constant output row
    acc += (b_o + b_v @ w_o).astype(np.float64)
    return acc.astype(np.float32).reshape(1, L, D)


# revision 15
# speedup vs baseline: 1.2304x; 1.2304x over previous
"""Multi-head attention (B=1, L=2048, D=1024, H=16) on 8 TRN2 NeuronCores.

Sharding: tensor-parallel over heads. Core i computes heads 2i, 2i+1:
  - projections with column shards of w_q/w_k/w_v (128 cols each)
  - full attention for its 2 heads
  - partial output projection with the matching 128-row shard of w_o
Host sums the 8 partial outputs and adds the fused bias b_o + b_v @ w_o
(b_v contributes a constant row to the output; b_k cancels in softmax).

Strip-pipelined schedule (all matmuls bf16, fp32 PSUM):
  - q processed in 4 strips of 512; per (strip, kt) iteration:
      S^T pair (row-tiled K=64 matmuls, heads at PE row groups 0/64)
      -> ONE exp over [128, 1024] (both heads, single PSUM tile)
      -> AV pair (col-tiled M=64, heads at PSUM partition groups 0/64)
      -> denominator accumulate split across VectorE / GpSimd by kt parity
  - tensor queue padded with out-projection chunks of strip s-1,
    q-projection of strip s+1, vh blocks, and normalize matmuls so the
    PE never idles (keeps the 2.4 GHz p-state)
  - host supplies q strip-major and v kt-major so DMA descriptors stay
    large and vh blocks become available incrementally
  - per-strip denominator reciprocal via partition-spread DMA
"""

import os
import numpy as np
import ml_dtypes

import concourse.bass as bass
import concourse.mybir as mybir
import concourse.tile as tile
from concourse import bacc
from concourse.bass import ts
from concourse.bass_utils import run_bass_kernel_spmd

P = 128
L = 2048
D = 1024
DH = 64
NCORES = 8
NSTRIP = 4
SW = 512  # strip width (q columns per strip)
KT = D // P  # 8 contraction tiles for the projections
LT = L // P  # 16 seq tiles
BF16 = mybir.dt.bfloat16
F32 = mybir.dt.float32
AF = mybir.ActivationFunctionType
ALU = mybir.AluOpType

TRACE = False  # test.py flips this to get an NTFF profile / exec_time_ns
LAST_RESULT = {}

_CACHED_NC = None


def _build():
    nc = bacc.Bacc("TRN2", target_bir_lowering=False, debug=False, num_devices=NCORES)

    kS = nc.dram_tensor("kS", [NSTRIP, P, KT, SW], BF16, kind="ExternalInput")
    qS = nc.dram_tensor("qS", [NSTRIP, P, KT, SW], BF16, kind="ExternalInput")
    vK = nc.dram_tensor("vK", [LT, P, KT, P], BF16, kind="ExternalInput")
    wq = nc.dram_tensor("wq", [P, KT, P], BF16, kind="ExternalInput")
    wk = nc.dram_tensor("wk", [P, KT, P], BF16, kind="ExternalInput")
    wv = nc.dram_tensor("wv", [P, KT, P], BF16, kind="ExternalInput")
    bq = nc.dram_tensor("bq", [P, 1], F32, kind="ExternalInput")
    wo = nc.dram_tensor("wo", [P, D], BF16, kind="ExternalInput")
    out = nc.dram_tensor("out", [P, LT, D], BF16, kind="ExternalOutput")

    with tile.TileContext(nc) as tc:
        with (
            tc.tile_pool(name="const", bufs=1) as const_pool,
            tc.tile_pool(name="inputs", bufs=1) as in_pool,
            tc.tile_pool(name="proj", bufs=1) as proj_pool,
            tc.tile_pool(name="work", bufs=1) as work_pool,
            tc.tile_pool(name="ps", bufs=1, space="PSUM") as psp,
            tc.tile_pool(name="ptp", bufs=3) as pt_pool,
            tc.tile_pool(name="accp", bufs=2) as acc_pool,
            tc.tile_pool(name="up", bufs=2) as u_pool,
            tc.tile_pool(name="osbp", bufs=2) as osb_pool,
        ):
            ones_c = const_pool.tile([P, P], BF16)
            nc.vector.memset(ones_c[:], 1.0)
            scr = const_pool.tile([1, 32], F32)
            nc.scalar.activation(scr[:], ones_c[0:1, 0:32], AF.Exp)

            # ---- input DMAs: weights, then k, then q strip0, v blocks,
            # then remaining q strips ----
            wq_sb = in_pool.tile([P, KT, P], BF16)
            wk_sb = in_pool.tile([P, KT, P], BF16)
            wv_sb = in_pool.tile([P, KT, P], BF16)
            bq_sb = in_pool.tile([P, 1], F32)
            wo_sb = in_pool.tile([P, D], BF16)
            kS_sb = in_pool.tile([P, NSTRIP, KT, SW], BF16)
            qS_sb = in_pool.tile([P, NSTRIP, KT, SW], BF16)
            vK_sb = in_pool.tile([P, LT, KT, P], BF16)
            # critical data first on every ring; weights (small, needed
            # slightly later) follow; the vK stream is paced by the loop
            nc.sync.dma_start(kS_sb[:, 0, 0:4, :], kS[0][:, 0:4, :])
            nc.gpsimd.dma_start(kS_sb[:, 0, 4:8, :], kS[0][:, 4:8, :])
            nc.scalar.dma_start(qS_sb[:, 0, :, :], qS[0])
            nc.sync.dma_start(wk_sb[:], wk[:])
            nc.scalar.dma_start(wq_sb[:], wq[:])
            nc.gpsimd.dma_start(wv_sb[:], wv[:])
            nc.scalar.dma_start(bq_sb[:], bq[:])
            nc.sync.dma_start(vK_sb[:, 0, :, :], vK[0])
            nc.gpsimd.dma_start(kS_sb[:, 1, :, :], kS[1])
            nc.scalar.dma_start(vK_sb[:, 1, :, :], vK[1])
            nc.sync.dma_start(vK_sb[:, 2, :, :], vK[2])
            nc.gpsimd.dma_start(wo_sb[:], wo[:])
            nc.scalar.dma_start(vK_sb[:, 3, :, :], vK[3])
            nc.sync.dma_start(vK_sb[:, 4, :, :], vK[4])
            nc.gpsimd.dma_start(vK_sb[:, 5, :, :], vK[5])
            nc.sync.dma_start(vK_sb[:, 6, :, :], vK[6])

            def paced_dma(s, kt):
                """Deferred input DMAs, triggered on the scalar queue right
                after exp(s, kt) so the stream is paced by loop progress."""
                if s == 0:
                    if kt <= 8:
                        nc.scalar.dma_start(vK_sb[:, kt + 7, :, :], vK[kt + 7])
                    if kt == 0:
                        nc.scalar.dma_start(kS_sb[:, 2, :, :], kS[2])
                    elif kt == 1:
                        nc.scalar.dma_start(kS_sb[:, 3, :, :], kS[3])
                    elif kt == 2:
                        nc.scalar.dma_start(qS_sb[:, 1, :, :], qS[1])
                elif s < NSTRIP - 1 and kt == 1:
                    nc.scalar.dma_start(qS_sb[:, s + 1, :, :], qS[s + 1])

            # ---- persistent SBUF tensors ----
            khT = proj_pool.tile([P, L], BF16)
            qhT = proj_pool.tile([P, L], BF16)
            vh_sb = proj_pool.tile([P, LT, P], BF16)  # [kseq, kt, dh-pair]
            lhsT_c = work_pool.tile([P, L], BF16)  # normalized concat^T
            rdb = work_pool.tile([1, NSTRIP, 2 * SW], BF16)  # 1/denominators

            def proj_k_chunk(n):
                """khT[:, n*512:(n+1)*512] (no bias: b_k cancels in softmax)."""
                ps = psp.tile([P, SW], F32, tag="mm", bufs=2, name=f"mmk_{n}")
                for t in range(KT):
                    nc.tensor.matmul(
                        ps[:],
                        wk_sb[:, t, :],
                        kS_sb[:, n, t, :],
                        start=(t == 0),
                        stop=(t == KT - 1),
                    )
                nc.scalar.copy(khT[:, ts(n, SW)], ps[:])

            _qps = {}

            def proj_q_part(s, half):
                """Half of the q-projection for strip s (pad-slot sized)."""
                if half == 0:
                    _qps[s] = psp.tile(
                        [P, SW], F32, tag="mm", bufs=2, name=f"mmq_{s}"
                    )
                ps = _qps[s]
                for t in range(4 * half, 4 * half + 4):
                    nc.tensor.matmul(
                        ps[:],
                        wq_sb[:, t, :],
                        qS_sb[:, s, t, :],
                        start=(t == 0),
                        stop=(t == KT - 1),
                    )
                if half == 1:
                    nc.vector.tensor_scalar(
                        qhT[:, ts(s, SW)], ps[:], bq_sb[:], None, op0=ALU.add
                    )

            def proj_q_strip(s):
                proj_q_part(s, 0)
                proj_q_part(s, 1)

            def vh_block(b):
                """vh_sb[:, b, :] = (v @ w_v)[b-th kseq tile] directly."""
                ps = psp.tile([P, SW], F32, tag="mm", bufs=2, name=f"mmv_{b}")
                for t in range(KT):
                    nc.tensor.matmul(
                        ps[:, 0:P],
                        vK_sb[:, b, t, :],
                        wv_sb[:, t, :],
                        start=(t == 0),
                        stop=(t == KT - 1),
                    )
                nc.vector.tensor_copy(vh_sb[:, b, :], ps[:, 0:P])

            # ---- prologue ----
            proj_k_chunk(0)
            proj_q_strip(0)

            # ---- strip-pipelined attention ----
            accs = [None] * NSTRIP
            us = [None] * NSTRIP
            dsps = [None] * NSTRIP
            osbs = {}

            dps3 = [None]

            def fin_a(s, acc_tail=None):
                """Denominator column-sums of strip s + spread DMA."""
                if acc_tail is not None:
                    dps = dps3[0]
                    for h in (0, 1):
                        nc.tensor.matmul(
                            dps[32 * h : 32 * h + 1, :],
                            ones_c[:, 0:1],
                            acc_tail[:, ts(h, SW)],
                            start=False,
                            stop=True,
                        )
                else:
                    acc_g, acc_v = accs[s]
                    nc.vector.tensor_tensor(
                        acc_v[:], acc_v[:], acc_g[:], op=ALU.add
                    )
                    dps = psp.tile(
                        [P, SW], F32, tag="mm", bufs=2, name=f"dcs_{s}"
                    )
                    for h in (0, 1):
                        nc.tensor.matmul(
                            dps[32 * h : 32 * h + 1, :],
                            ones_c[:, 0:1],
                            acc_v[:, ts(h, SW)],
                        )
                dsb = work_pool.tile(
                    [1, 2 * SW], F32, tag="dsb", bufs=2, name=f"dsb_{s}"
                )
                nc.scalar.copy(dsb[0:1, 0:SW], dps[0:1, :])
                nc.scalar.copy(dsb[0:1, SW : 2 * SW], dps[32:33, :])
                dsp = work_pool.tile([P, 8], F32, tag="dsp", bufs=2, name=f"dsp_{s}")
                dsps[s] = dsp
                nc.sync.dma_start(dsp[0:DH, :], dsb[0:1, 0:SW])
                nc.gpsimd.dma_start(dsp[DH:P, :], dsb[0:1, SW : 2 * SW])

            def fin_b(s):
                """Reciprocal on the spread layout + gather back."""
                dsp = dsps[s]
                nc.vector.reciprocal(dsp[:], dsp[:])
                dspb = work_pool.tile([P, 8], BF16, tag="dspb", bufs=2, name=f"dspb_{s}")
                nc.vector.tensor_copy(dspb[:], dsp[:])
                nc.sync.dma_start(rdb[0:1, s, 0:SW], dspb[0:DH, :])
                nc.gpsimd.dma_start(rdb[0:1, s, SW : 2 * SW], dspb[DH:P, :])

            def fin_c(s):
                """Broadcast 1/d over partitions, normalize -> lhsT_c."""
                bc = psp.tile([P, SW], F32, tag="mm", bufs=2, name=f"bc_{s}")
                for h in (0, 1):
                    nc.tensor.matmul(
                        bc[ts(h, DH), :],
                        ones_c[0:1, 0:DH],
                        rdb[0:1, s, ts(h, SW)],
                    )
                for m4 in range(4):
                    nc.vector.tensor_tensor(
                        lhsT_c[:, 4 * s * P + m4 * P : 4 * s * P + (m4 + 1) * P],
                        us[s][:, ts(m4, P)],
                        bc[:, ts(m4, P)],
                        op=ALU.mult,
                    )

            def outproj_chunk(s, m, n, cp_eng="v"):
                """Partial out-projection chunk (q subtile m, D half n) of
                strip s into the per-strip osb block; one big DMA at the
                end of each strip (8 KB descriptors via the blocked out)."""
                ps = psp.tile([P, SW], F32, tag="mm", bufs=2, name=f"op_{s}_{m}_{n}")
                nc.tensor.matmul(
                    ps[:], lhsT_c[:, ts(4 * s + m, P)], wo_sb[:, ts(n, SW)]
                )
                osb = osbs.get(s)
                if osb is None:
                    osb = osb_pool.tile([P, 4, D], BF16, tag="osb", name=f"osb_{s}")
                    osbs[s] = osb
                (nc.scalar.copy if cp_eng == "s" else nc.vector.tensor_copy)(
                    osb[:, m, ts(n, SW)], ps[:]
                )
                if m == 3 and n == 1:
                    (nc.sync if s % 2 == 0 else nc.gpsimd).dma_start(
                        out[:, 4 * s : 4 * s + 4, :], osb[:]
                    )

            for s in range(NSTRIP):
                av = psp.tile([P, SW], F32, tag="av", bufs=2, name=f"av_{s}")
                acc_g = acc_pool.tile([P, 2 * SW], BF16, tag="accg", name=f"accg_{s}")
                acc_v = acc_pool.tile([P, 2 * SW], BF16, tag="accv", name=f"accv_{s}")
                last = s == NSTRIP - 1
                if last:
                    acc_t = acc_pool.tile(
                        [P, 2 * SW], BF16, tag="acct", name=f"acct_{s}"
                    )
                accs[s] = (acc_g, acc_v)
                pts = [None, None, None]
                for kt in range(LT):
                    # scores: both heads into one PSUM tile (row groups 0/64)
                    st = psp.tile(
                        [P, 2 * SW], F32, tag="st", bufs=2, name=f"st_{s}_{kt}"
                    )
                    for h in (0, 1):
                        nc.tensor.matmul(
                            st[:, ts(h, SW)],
                            khT[ts(h, DH), ts(kt, P)],
                            qhT[ts(h, DH), ts(s, SW)],
                        )
                    # one exp for both heads; scale 1/sqrt(64) folded in
                    pt = pt_pool.tile([P, 2 * SW], BF16, tag="pt", name=f"pt_{s}_{kt}")
                    pts[kt % 3] = pt
                    nc.scalar.activation(pt[:], st[:], AF.Exp, scale=0.125)
                    paced_dma(s, kt)

                    # tensor pad work (keeps PE busy while exp(kt) finishes)
                    if s == 0:
                        if kt == 0:
                            vh_block(0)
                            vh_block(1)
                            vh_block(2)
                        elif kt <= 13:
                            vh_block(kt + 2)
                        elif kt == 14:
                            proj_q_part(1, 0)
                        elif kt == 15:
                            proj_q_part(1, 1)
                        if kt == 1:
                            proj_k_chunk(1)
                        elif kt == 5:
                            proj_k_chunk(2)
                        elif kt == 9:
                            proj_k_chunk(3)
                    else:
                        if kt == 0:
                            fin_a(s - 1)
                        elif kt == 3:
                            fin_b(s - 1)
                        elif kt == 5:
                            fin_c(s - 1)
                        elif 6 <= kt <= 13:
                            outproj_chunk(s - 1, (kt - 6) // 2, (kt - 6) % 2)
                        elif kt == 14 and s + 1 < NSTRIP:
                            proj_q_part(s + 1, 0)
                        elif kt == 15 and s + 1 < NSTRIP:
                            proj_q_part(s + 1, 1)

                    # AV pair of the previous kt (exp long since done)
                    def av_pair(k):
                        for h in (0, 1):
                            nc.tensor.matmul(
                                av[ts(h, DH), :],
                                vh_sb[:, k, ts(h, DH)],
                                pts[k % 3][:, ts(h, SW)],
                                start=(k == 0),
                                stop=(k == LT - 1),
                            )

                    if kt > 0:
                        av_pair(kt - 1)
                    # two independent denominator chains; gpsimd (slow per
                    # op) only gets mid-strip slots so it never lags fin_a.
                    # On the last strip, kts 12-15 go to a third chain so
                    # the main chains can be column-summed early (kt 13).
                    if last and kt >= 12:
                        if kt == 12:
                            nc.vector.tensor_copy(acc_t[:], pt[:])
                        else:
                            nc.vector.tensor_tensor(
                                acc_t[:], acc_t[:], pt[:], op=ALU.add
                            )
                    elif kt in (2, 5, 8, 11):
                        if kt == 2:
                            nc.gpsimd.tensor_copy(acc_g[:], pt[:])
                        else:
                            nc.gpsimd.tensor_tensor(
                                acc_g[:], acc_g[:], pt[:], op=ALU.add
                            )
                    else:
                        if kt == 0:
                            nc.vector.tensor_copy(acc_v[:], pt[:])
                        else:
                            nc.vector.tensor_tensor(
                                acc_v[:], acc_v[:], pt[:], op=ALU.add
                            )
                    if last and kt == 13:
                        nc.vector.tensor_tensor(
                            acc_v[:], acc_v[:], acc_g[:], op=ALU.add
                        )
                        dps3[0] = psp.tile(
                            [P, SW], F32, tag="mm", bufs=2, name="dcs_3"
                        )
                        for h in (0, 1):
                            nc.tensor.matmul(
                                dps3[0][32 * h : 32 * h + 1, :],
                                ones_c[:, 0:1],
                                acc_v[:, ts(h, SW)],
                                start=True,
                                stop=False,
                            )
                av_pair(LT - 1)
                # unnormalized attention out of this strip -> SBUF
                u = u_pool.tile([P, SW], F32, tag="u", name=f"u_{s}")
                us[s] = u
                nc.scalar.copy(u[:], av[:])

            # ---- epilogue for the last strip ----
            s = NSTRIP - 1
            fin_a(s, acc_tail=acc_t)
            fin_b(s)
            fin_c(s)
            for m in range(4):
                for n in range(2):
                    outproj_chunk(s, m, n, cp_eng="s" if (2 * m + n) % 2 else "v")

    nc.compile()
    return nc


def kernel(q, k, v, w_q, b_q, w_k, b_k, w_v, b_v, w_o, b_o):
    global _CACHED_NC, LAST_RESULT
    if _CACHED_NC is None:
        _CACHED_NC = _build()
    nc = _CACHED_NC

    bf16 = ml_dtypes.bfloat16

    def tile_T(x):  # [L, D] -> [128, D//128, L] contiguous
        xt = np.asarray(x, np.float32)[0].T  # [D, L]
        return np.ascontiguousarray(
            xt.reshape(D // P, P, L).transpose(1, 0, 2)
        ).astype(bf16)

    def tile_w(w):  # [D, 128] -> [128, D//128, 128] contiguous
        return np.ascontiguousarray(
            w.reshape(D // P, P, P).transpose(1, 0, 2)
        ).astype(bf16)

    # k and q strip-major: [NSTRIP, 128, KT, 512]
    k2 = np.ascontiguousarray(
        tile_T(k).reshape(P, KT, NSTRIP, SW).transpose(2, 0, 1, 3)
    )
    q2 = np.ascontiguousarray(
        tile_T(q).reshape(P, KT, NSTRIP, SW).transpose(2, 0, 1, 3)
    )
    # v kt-major: [LT, 128, KT, 128]
    v2 = np.ascontiguousarray(
        tile_T(v).reshape(P, KT, LT, P).transpose(2, 0, 1, 3)
    )
    w_q = np.asarray(w_q, np.float32)
    w_k = np.asarray(w_k, np.float32)
    w_v = np.asarray(w_v, np.float32)
    w_o = np.asarray(w_o, np.float32)
    b_q = np.asarray(b_q, np.float32)
    b_v = np.asarray(b_v, np.float32)
    b_o = np.asarray(b_o, np.float32)

    in_maps = []
    for i in range(NCORES):
        sl = slice(P * i, P * (i + 1))
        in_maps.append(
            {
                "kS": k2,
                "qS": q2,
                "vK": v2,
                "wq": tile_w(w_q[:, sl]),
                "wk": tile_w(w_k[:, sl]),
                "wv": tile_w(w_v[:, sl]),
                "bq": np.ascontiguousarray(b_q[sl]).reshape(P, 1),
                "wo": np.ascontiguousarray(w_o[sl, :]).astype(bf16),
            }
        )

    kwargs = {}
    if TRACE:
        import shutil

        tdir = "/tmp/bass_trace"
        shutil.rmtree(tdir, ignore_errors=True)
        os.makedirs(tdir, exist_ok=True)
        kwargs["tmpdir"] = tdir
    res = run_bass_kernel_spmd(nc, in_maps, list(range(NCORES)), trace=TRACE, **kwargs)
    LAST_RESULT = {
        "exec_time_ns": res.exec_time_ns,
        "trace_path": (res.instructions_and_trace or (None, None))[1],
    }
    acc = np.zeros((L, D), np.float64)
    for i in range(NCORES):
        ob = res.results[i]["out"]  # [P, LT, D] blocked
        acc += ob.transpose(1, 0, 2).reshape(L, D).astype(np.float64)
    # b_k cancels in softmax; b_v and b_o contribute a constant output row
    acc += (b_o + b_v @ w_o).astype(np.float64)
    return acc.astype(np.float32).reshape(1, L, D)


# revision 16
# speedup vs baseline: 1.2657x; 1.0287x over previous
"""Multi-head attention (B=1, L=2048, D=1024, H=16) on 8 TRN2 NeuronCores.

Sharding: tensor-parallel over heads. Core i computes heads 2i, 2i+1:
  - projections with column shards of w_q/w_k/w_v (128 cols each)
  - full attention for its 2 heads
  - partial output projection with the matching 128-row shard of w_o
Host sums the 8 partial outputs and adds the fused bias b_o + b_v @ w_o
(b_v contributes a constant row to the output; b_k cancels in softmax).

Strip-pipelined schedule (all matmuls bf16, fp32 PSUM):
  - q processed in 4 strips of 512; per (strip, kt) iteration:
      S^T pair (row-tiled K=64 matmuls, heads at PE row groups 0/64)
      -> ONE exp over [128, 1024] (both heads, single PSUM tile)
      -> AV pair (col-tiled M=64, heads at PSUM partition groups 0/64)
      -> denominator accumulate split across VectorE / GpSimd by kt parity
  - tensor queue padded with out-projection chunks of strip s-1,
    q-projection of strip s+1, vh blocks, and normalize matmuls so the
    PE never idles (keeps the 2.4 GHz p-state)
  - host supplies q strip-major and v kt-major so DMA descriptors stay
    large and vh blocks become available incrementally
  - per-strip denominator reciprocal via partition-spread DMA
"""

import os
import numpy as np
import ml_dtypes

import concourse.bass as bass
import concourse.mybir as mybir
import concourse.tile as tile
from concourse import bacc
from concourse.bass import ts
from concourse.bass_utils import run_bass_kernel_spmd

P = 128
L = 2048
D = 1024
DH = 64
NCORES = 8
NSTRIP = 4
SW = 512  # strip width (q columns per strip)
KT = D // P  # 8 contraction tiles for the projections
LT = L // P  # 16 seq tiles
BF16 = mybir.dt.bfloat16
F32 = mybir.dt.float32
AF = mybir.ActivationFunctionType
ALU = mybir.AluOpType

TRACE = False  # test.py flips this to get an NTFF profile / exec_time_ns
LAST_RESULT = {}

_CACHED_NC = None


def _build():
    nc = bacc.Bacc("TRN2", target_bir_lowering=False, debug=False, num_devices=NCORES)

    kS = nc.dram_tensor("kS", [NSTRIP, P, KT, SW], BF16, kind="ExternalInput")
    qS = nc.dram_tensor("qS", [NSTRIP, P, KT, SW], BF16, kind="ExternalInput")
    vK = nc.dram_tensor("vK", [LT, P, KT, P], BF16, kind="ExternalInput")
    wq = nc.dram_tensor("wq", [P, KT, P], BF16, kind="ExternalInput")
    wk = nc.dram_tensor("wk", [P, KT, P], BF16, kind="ExternalInput")
    wv = nc.dram_tensor("wv", [P, KT, P], BF16, kind="ExternalInput")
    bq = nc.dram_tensor("bq", [P, 1], F32, kind="ExternalInput")
    wo = nc.dram_tensor("wo", [P, D], BF16, kind="ExternalInput")
    out = nc.dram_tensor("out", [P, LT, D], BF16, kind="ExternalOutput")

    with tile.TileContext(nc) as tc:
        with (
            tc.tile_pool(name="const", bufs=1) as const_pool,
            tc.tile_pool(name="inputs", bufs=1) as in_pool,
            tc.tile_pool(name="proj", bufs=1) as proj_pool,
            tc.tile_pool(name="work", bufs=1) as work_pool,
            tc.tile_pool(name="ps", bufs=1, space="PSUM") as psp,
            tc.tile_pool(name="ptp", bufs=3) as pt_pool,
            tc.tile_pool(name="accp", bufs=2) as acc_pool,
            tc.tile_pool(name="up", bufs=2) as u_pool,
            tc.tile_pool(name="osbp", bufs=2) as osb_pool,
        ):
            ones_c = const_pool.tile([P, P], BF16)
            nc.vector.memset(ones_c[:], 1.0)
            scr = const_pool.tile([1, 32], F32)
            nc.scalar.activation(scr[:], ones_c[0:1, 0:32], AF.Exp)

            # ---- input DMAs: weights, then k, then q strip0, v blocks,
            # then remaining q strips ----
            wq_sb = in_pool.tile([P, KT, P], BF16)
            wk_sb = in_pool.tile([P, KT, P], BF16)
            wv_sb = in_pool.tile([P, KT, P], BF16)
            bq_sb = in_pool.tile([P, 1], F32)
            wo_sb = in_pool.tile([P, D], BF16)
            kS_sb = in_pool.tile([P, NSTRIP, KT, SW], BF16)
            qS_sb = in_pool.tile([P, NSTRIP, KT, SW], BF16)
            vK_sb = in_pool.tile([P, LT, KT, P], BF16)
            # critical data first on every ring; weights (small, needed
            # slightly later) follow; the vK stream is paced by the loop
            nc.sync.dma_start(kS_sb[:, 0, 0:4, :], kS[0][:, 0:4, :])
            nc.gpsimd.dma_start(kS_sb[:, 0, 4:8, :], kS[0][:, 4:8, :])
            nc.scalar.dma_start(qS_sb[:, 0, :, :], qS[0])
            nc.sync.dma_start(wk_sb[:], wk[:])
            nc.scalar.dma_start(wq_sb[:], wq[:])
            nc.gpsimd.dma_start(wv_sb[:], wv[:])
            nc.scalar.dma_start(bq_sb[:], bq[:])
            nc.sync.dma_start(vK_sb[:, 0, :, :], vK[0])
            nc.gpsimd.dma_start(kS_sb[:, 1, :, :], kS[1])
            nc.scalar.dma_start(vK_sb[:, 1, :, :], vK[1])
            nc.sync.dma_start(vK_sb[:, 2, :, :], vK[2])
            nc.gpsimd.dma_start(wo_sb[:], wo[:])
            nc.scalar.dma_start(vK_sb[:, 3, :, :], vK[3])
            nc.sync.dma_start(vK_sb[:, 4, :, :], vK[4])
            nc.gpsimd.dma_start(vK_sb[:, 5, :, :], vK[5])
            nc.sync.dma_start(vK_sb[:, 6, :, :], vK[6])

            def paced_dma(s, kt):
                """Deferred input DMAs, triggered on the scalar queue right
                after exp(s, kt) so the stream is paced by loop progress."""
                if s == 0:
                    if kt <= 8:
                        nc.scalar.dma_start(vK_sb[:, kt + 7, :, :], vK[kt + 7])
                    if kt == 0:
                        nc.scalar.dma_start(kS_sb[:, 2, :, :], kS[2])
                    elif kt == 1:
                        nc.scalar.dma_start(kS_sb[:, 3, :, :], kS[3])
                    elif kt == 2:
                        nc.scalar.dma_start(qS_sb[:, 1, :, :], qS[1])
                elif s < NSTRIP - 1 and kt == 1:
                    nc.scalar.dma_start(qS_sb[:, s + 1, :, :], qS[s + 1])

            # ---- persistent SBUF tensors ----
            khT = proj_pool.tile([P, L], BF16)
            qhT = proj_pool.tile([P, L], BF16)
            vh_sb = proj_pool.tile([P, LT, P], BF16)  # [kseq, kt, dh-pair]
            lhsT_c = work_pool.tile([P, L], BF16)  # normalized concat^T
            rdb = work_pool.tile([1, NSTRIP, 2 * SW], BF16)  # 1/denominators

            def proj_k_chunk(n):
                """khT[:, n*512:(n+1)*512] (no bias: b_k cancels in softmax)."""
                ps = psp.tile([P, SW], F32, tag="mm", bufs=2, name=f"mmk_{n}")
                for t in range(KT):
                    nc.tensor.matmul(
                        ps[:],
                        wk_sb[:, t, :],
                        kS_sb[:, n, t, :],
                        start=(t == 0),
                        stop=(t == KT - 1),
                    )
                nc.scalar.copy(khT[:, ts(n, SW)], ps[:])

            _qps = {}

            def proj_q_part(s, half):
                """Half of the q-projection for strip s (pad-slot sized)."""
                if half == 0:
                    _qps[s] = psp.tile(
                        [P, SW], F32, tag="mm", bufs=2, name=f"mmq_{s}"
                    )
                ps = _qps[s]
                for t in range(4 * half, 4 * half + 4):
                    nc.tensor.matmul(
                        ps[:],
                        wq_sb[:, t, :],
                        qS_sb[:, s, t, :],
                        start=(t == 0),
                        stop=(t == KT - 1),
                    )
                if half == 1:
                    nc.vector.tensor_scalar(
                        qhT[:, ts(s, SW)], ps[:], bq_sb[:], None, op0=ALU.add
                    )

            def proj_q_strip(s):
                proj_q_part(s, 0)
                proj_q_part(s, 1)

            def vh_block(b):
                """vh_sb[:, b, :] = (v @ w_v)[b-th kseq tile] directly."""
                ps = psp.tile([P, SW], F32, tag="mm", bufs=2, name=f"mmv_{b}")
                for t in range(KT):
                    nc.tensor.matmul(
                        ps[:, 0:P],
                        vK_sb[:, b, t, :],
                        wv_sb[:, t, :],
                        start=(t == 0),
                        stop=(t == KT - 1),
                    )
                nc.vector.tensor_copy(vh_sb[:, b, :], ps[:, 0:P])

            # ---- prologue ----
            proj_k_chunk(0)
            proj_q_strip(0)

            # ---- strip-pipelined attention ----
            accs = [None] * NSTRIP
            us = [None] * NSTRIP
            dsps = [None] * NSTRIP
            osbs = {}

            dps3 = [None]

            def fin_a(s, acc_tail=None):
                """Denominator column-sums of strip s + spread DMA."""
                if acc_tail is not None:
                    dps = dps3[0]
                    for h in (0, 1):
                        nc.tensor.matmul(
                            dps[32 * h : 32 * h + 1, :],
                            ones_c[:, 0:1],
                            acc_tail[:, ts(h, SW)],
                            start=False,
                            stop=True,
                        )
                else:
                    acc_g, acc_v = accs[s]
                    nc.vector.tensor_tensor(
                        acc_v[:], acc_v[:], acc_g[:], op=ALU.add
                    )
                    dps = psp.tile(
                        [P, SW], F32, tag="mm", bufs=2, name=f"dcs_{s}"
                    )
                    for h in (0, 1):
                        nc.tensor.matmul(
                            dps[32 * h : 32 * h + 1, :],
                            ones_c[:, 0:1],
                            acc_v[:, ts(h, SW)],
                        )
                dsps[s] = dps

            def fin_b(s):
                """Direct reciprocals into a staging row, then bf16."""
                dps = dsps[s]
                rdf = work_pool.tile(
                    [1, 2 * SW], F32, tag="rdf", bufs=2, name=f"rdf_{s}"
                )
                nc.vector.reciprocal(rdf[0:1, 0:SW], dps[0:1, :])
                nc.vector.reciprocal(rdf[0:1, SW : 2 * SW], dps[32:33, :])
                nc.vector.tensor_copy(rdb[0:1, s, :], rdf[0:1, :])

            def fin_c(s):
                """Broadcast 1/d over partitions, normalize -> lhsT_c."""
                bc = psp.tile([P, SW], F32, tag="mm", bufs=2, name=f"bc_{s}")
                for h in (0, 1):
                    nc.tensor.matmul(
                        bc[ts(h, DH), :],
                        ones_c[0:1, 0:DH],
                        rdb[0:1, s, ts(h, SW)],
                    )
                for m4 in range(4):
                    nc.vector.tensor_tensor(
                        lhsT_c[:, 4 * s * P + m4 * P : 4 * s * P + (m4 + 1) * P],
                        us[s][:, ts(m4, P)],
                        bc[:, ts(m4, P)],
                        op=ALU.mult,
                    )

            def outproj_chunk(s, m, n, cp_eng="v"):
                """Partial out-projection chunk (q subtile m, D half n) of
                strip s into the per-strip osb block; one big DMA at the
                end of each strip (8 KB descriptors via the blocked out)."""
                ps = psp.tile([P, SW], F32, tag="mm", bufs=2, name=f"op_{s}_{m}_{n}")
                nc.tensor.matmul(
                    ps[:], lhsT_c[:, ts(4 * s + m, P)], wo_sb[:, ts(n, SW)]
                )
                osb = osbs.get(s)
                if osb is None:
                    osb = osb_pool.tile([P, 4, D], BF16, tag="osb", name=f"osb_{s}")
                    osbs[s] = osb
                (nc.scalar.copy if cp_eng == "s" else nc.vector.tensor_copy)(
                    osb[:, m, ts(n, SW)], ps[:]
                )
                if m == 3 and n == 1:
                    (nc.sync if s % 2 == 0 else nc.gpsimd).dma_start(
                        out[:, 4 * s : 4 * s + 4, :], osb[:]
                    )

            for s in range(NSTRIP):
                av = psp.tile([P, SW], F32, tag="av", bufs=2, name=f"av_{s}")
                acc_g = acc_pool.tile([P, 2 * SW], BF16, tag="accg", name=f"accg_{s}")
                acc_v = acc_pool.tile([P, 2 * SW], BF16, tag="accv", name=f"accv_{s}")
                last = s == NSTRIP - 1
                if last:
                    acc_t = acc_pool.tile(
                        [P, 2 * SW], BF16, tag="acct", name=f"acct_{s}"
                    )
                accs[s] = (acc_g, acc_v)
                pts = [None, None, None]
                for kt in range(LT):
                    # scores: both heads into one PSUM tile (row groups 0/64)
                    st = psp.tile(
                        [P, 2 * SW], F32, tag="st", bufs=2, name=f"st_{s}_{kt}"
                    )
                    for h in (0, 1):
                        nc.tensor.matmul(
                            st[:, ts(h, SW)],
                            khT[ts(h, DH), ts(kt, P)],
                            qhT[ts(h, DH), ts(s, SW)],
                        )
                    # one exp for both heads; scale 1/sqrt(64) folded in
                    pt = pt_pool.tile([P, 2 * SW], BF16, tag="pt", name=f"pt_{s}_{kt}")
                    pts[kt % 3] = pt
                    nc.scalar.activation(pt[:], st[:], AF.Exp, scale=0.125)
                    paced_dma(s, kt)

                    # tensor pad work (keeps PE busy while exp(kt) finishes)
                    if s == 0:
                        if kt == 0:
                            vh_block(0)
                            vh_block(1)
                            vh_block(2)
                        elif kt <= 13:
                            vh_block(kt + 2)
                        elif kt == 14:
                            proj_q_part(1, 0)
                        elif kt == 15:
                            proj_q_part(1, 1)
                        if kt == 1:
                            proj_k_chunk(1)
                        elif kt == 5:
                            proj_k_chunk(2)
                        elif kt == 9:
                            proj_k_chunk(3)
                    else:
                        if kt == 0:
                            fin_a(s - 1)
                        elif kt == 2:
                            fin_b(s - 1)
                        elif kt == 4:
                            fin_c(s - 1)
                        elif 6 <= kt <= 13:
                            outproj_chunk(s - 1, (kt - 6) // 2, (kt - 6) % 2)
                        elif kt == 14 and s + 1 < NSTRIP:
                            proj_q_part(s + 1, 0)
                        elif kt == 15 and s + 1 < NSTRIP:
                            proj_q_part(s + 1, 1)

                    # AV pair of the previous kt (exp long since done)
                    def av_pair(k):
                        for h in (0, 1):
                            nc.tensor.matmul(
                                av[ts(h, DH), :],
                                vh_sb[:, k, ts(h, DH)],
                                pts[k % 3][:, ts(h, SW)],
                                start=(k == 0),
                                stop=(k == LT - 1),
                            )

                    if kt > 0:
                        av_pair(kt - 1)
                    # two independent denominator chains; gpsimd (slow per
                    # op) only gets mid-strip slots so it never lags fin_a.
                    # On the last strip, kts 12-15 go to a third chain so
                    # the main chains can be column-summed early (kt 13).
                    if last and kt >= 12:
                        if kt == 12:
                            nc.vector.tensor_copy(acc_t[:], pt[:])
                        else:
                            nc.vector.tensor_tensor(
                                acc_t[:], acc_t[:], pt[:], op=ALU.add
                            )
                    elif kt in (2, 5, 8, 11):
                        if kt == 2:
                            nc.gpsimd.tensor_copy(acc_g[:], pt[:])
                        else:
                            nc.gpsimd.tensor_tensor(
                                acc_g[:], acc_g[:], pt[:], op=ALU.add
                            )
                    else:
                        if kt == 0:
                            nc.vector.tensor_copy(acc_v[:], pt[:])
                        else:
                            nc.vector.tensor_tensor(
                                acc_v[:], acc_v[:], pt[:], op=ALU.add
                            )
                    if last and kt == 13:
                        nc.vector.tensor_tensor(
                            acc_v[:], acc_v[:], acc_g[:], op=ALU.add
                        )
                        dps3[0] = psp.tile(
                            [P, SW], F32, tag="mm", bufs=2, name="dcs_3"
                        )
                        for h in (0, 1):
                            nc.tensor.matmul(
                                dps3[0][32 * h : 32 * h + 1, :],
                                ones_c[:, 0:1],
                                acc_v[:, ts(h, SW)],
                                start=True,
                                stop=False,
                            )
                av_pair(LT - 1)
                # unnormalized attention out of this strip -> SBUF
                u = u_pool.tile([P, SW], F32, tag="u", name=f"u_{s}")
                us[s] = u
                nc.scalar.copy(u[:], av[:])

            # ---- epilogue for the last strip: wide chunks, st tag is
            # free now; split the final store into two DMAs ----
            s = NSTRIP - 1
            fin_a(s, acc_tail=acc_t)
            fin_b(s)
            fin_c(s)
            osb = osb_pool.tile([P, 4, D], BF16, tag="osb", name="osb_3")
            for m in range(4):
                ps = psp.tile([P, 2 * SW], F32, tag="st", bufs=2, name=f"ope_{m}")
                for n in range(2):
                    nc.tensor.matmul(
                        ps[:, ts(n, SW)],
                        lhsT_c[:, ts(4 * s + m, P)],
                        wo_sb[:, ts(n, SW)],
                    )
                (nc.scalar.copy if m % 2 else nc.vector.tensor_copy)(
                    osb[:, m, :], ps[:]
                )
                if m == 1:
                    nc.sync.dma_start(out[:, 12:14, :], osb[:, 0:2, :])
                elif m == 3:
                    nc.gpsimd.dma_start(out[:, 14:16, :], osb[:, 2:4, :])

    nc.compile()
    return nc


def kernel(q, k, v, w_q, b_q, w_k, b_k, w_v, b_v, w_o, b_o):
    global _CACHED_NC, LAST_RESULT
    if _CACHED_NC is None:
        _CACHED_NC = _build()
    nc = _CACHED_NC

    bf16 = ml_dtypes.bfloat16

    def tile_T(x):  # [L, D] -> [128, D//128, L] contiguous
        xt = np.asarray(x, np.float32)[0].T  # [D, L]
        return np.ascontiguousarray(
            xt.reshape(D // P, P, L).transpose(1, 0, 2)
        ).astype(bf16)

    def tile_w(w):  # [D, 128] -> [128, D//128, 128] contiguous
        return np.ascontiguousarray(
            w.reshape(D // P, P, P).transpose(1, 0, 2)
        ).astype(bf16)

    # k and q strip-major: [NSTRIP, 128, KT, 512]
    k2 = np.ascontiguousarray(
        tile_T(k).reshape(P, KT, NSTRIP, SW).transpose(2, 0, 1, 3)
    )
    q2 = np.ascontiguousarray(
        tile_T(q).reshape(P, KT, NSTRIP, SW).transpose(2, 0, 1, 3)
    )
    # v kt-major: [LT, 128, KT, 128]
    v2 = np.ascontiguousarray(
        tile_T(v).reshape(P, KT, LT, P).transpose(2, 0, 1, 3)
    )
    w_q = np.asarray(w_q, np.float32)
    w_k = np.asarray(w_k, np.float32)
    w_v = np.asarray(w_v, np.float32)
    w_o = np.asarray(w_o, np.float32)
    b_q = np.asarray(b_q, np.float32)
    b_v = np.asarray(b_v, np.float32)
    b_o = np.asarray(b_o, np.float32)

    in_maps = []
    for i in range(NCORES):
        sl = slice(P * i, P * (i + 1))
        in_maps.append(
            {
                "kS": k2,
                "qS": q2,
                "vK": v2,
                "wq": tile_w(w_q[:, sl]),
                "wk": tile_w(w_k[:, sl]),
                "wv": tile_w(w_v[:, sl]),
                "bq": np.ascontiguousarray(b_q[sl]).reshape(P, 1),
                "wo": np.ascontiguousarray(w_o[sl, :]).astype(bf16),
            }
        )

    kwargs = {}
    if TRACE:
        import shutil

        tdir = "/tmp/bass_trace"
        shutil.rmtree(tdir, ignore_errors=True)
        os.makedirs(tdir, exist_ok=True)
        kwargs["tmpdir"] = tdir
    res = run_bass_kernel_spmd(nc, in_maps, list(range(NCORES)), trace=TRACE, **kwargs)
    LAST_RESULT = {
        "exec_time_ns": res.exec_time_ns,
        "trace_path": (res.instructions_and_trace or (None, None))[1],
    }
    acc = np.zeros((L, D), np.float64)
    for i in range(NCORES):
        ob = res.results[i]["out"]  # [P, LT, D] blocked
        acc += ob.transpose(1, 0, 2).reshape(L, D).astype(np.float64)
    # b_k cancels in softmax; b_v and b_o contribute a constant output row
    acc += (b_o + b_v @ w_o).astype(np.float64)
    return acc.astype(np.float32).reshape(1, L, D)
